# revision 6
# baseline (speedup 1.0000x reference)
"""Multi-head attention (shared QKV projection, floor-div scores) on 8 NeuronCores.

Problem: B=2, S=2048, HID=1024, NH=16, HD=64
    q = k = v = x @ Wq + bq          (reshaped to heads)
    scores = floor(q k^T / sqrt(64)) ; attn = softmax(scores)
    out = (attn v) @ Wo + bo

Sharding: core c handles batch c//4 and 4 heads ((c%4)*4 ..+4). Each core
computes its heads' contribution to out[b] = attn_out @ Wo; the host sums the
4 partials per batch and adds bo.

Device algorithm per core (all fp32):
  - host pre-scales xT by 8^-0.5 (and bq by 8^-0.5, Wo by 8^0.5) so the PE
    score matmuls directly produce s/8 = scores/sqrt(HD).
  - qT[2 head-pair tiles, 128x2048] = Wq_pair^T @ xT   (PE, PSUM accum)
  - v tiles = PE-transposed qT slices (+ ones column for rowsum Z)
  - per head: scoresT blocks [128k x 2048q] (PE) ->
      floor via RNE trick on DVE: n = (s/8 + 63.5) + 2^23  ->
      P = exp(n - (2^23+64)) on ACT ->
      oT[65, 2048] += v_i^T @ P_i  (PE, ones col gives Z row)
    then rz = exp(-ln(Z)) (ACT), broadcast via DRAM round-trip DMA,
    oTn = oT * rz (DVE)
  - partial = oTn_pair^T @ Wo_pair (PE, K=128 per pair) -> DMA PSUM->DRAM
"""

import math
import sys

sys.path.insert(0, "/opt/trn_rl_repo")

import numpy as np
import concourse.bass as bass
import concourse.bacc as bacc
import concourse.tile as tile
from concourse import mybir
from concourse.bass_utils import run_bass_kernel_spmd

F32 = mybir.dt.float32
ADD = mybir.AluOpType.add
MULT = mybir.AluOpType.mult
AF = mybir.ActivationFunctionType

B, S, HID, NH, HD = 2, 2048, 1024, 16, 64
HPC = 4          # heads per core
NCORES = 8
KT = HID // 128  # 8 k-tiles
QT = S // 128    # 16 q/s tiles
C23 = float(2 ** 23)
SQ8 = 1.0 / math.sqrt(8.0)

_NC_CACHE = None


def _build():
    nc = bacc.Bacc("TRN2", target_bir_lowering=False, debug=False,
                   num_devices=NCORES)

    xt = nc.dram_tensor("xt", [HID, S], F32, kind="ExternalInput")
    wq = [nc.dram_tensor(f"wq{p}", [128, 1024], F32, kind="ExternalInput")
          for p in range(2)]
    wo = [nc.dram_tensor(f"wo{p}", [128, 1024], F32, kind="ExternalInput")
          for p in range(2)]
    bq = [nc.dram_tensor(f"bq{p}", [128, 1], F32, kind="ExternalInput")
          for p in range(2)]
    ident = nc.dram_tensor("ident", [128, 64], F32, kind="ExternalInput")
    part = nc.dram_tensor("part", [S, HID], F32, kind="ExternalOutput")
    rzscr = nc.dram_tensor("rzscr", [HPC, S], F32)

    with tile.TileContext(nc) as tc:
        with (
            tc.tile_pool(name="cst", bufs=1) as cst,
            tc.tile_pool(name="big", bufs=1) as big,
            tc.tile_pool(name="wrk", bufs=2) as wrk,
            tc.tile_pool(name="zs", bufs=1) as zs,
            tc.tile_pool(name="ps_big", bufs=1, space="PSUM") as ps_big,
            tc.tile_pool(name="ps_small", bufs=2, space="PSUM") as ps_small,
        ):
            # ---- constants / inputs ----
            b_negc = cst.tile([128, 1], F32, tag="b_negc")
            nc.vector.memset(b_negc[:], -(C23 + 64.0))
            id_t = cst.tile([128, 64], F32, tag="id_t")
            nc.sync.dma_start(id_t[:], ident[:])
            wq_t, wo_t, bq_t = [], [], []
            for p in range(2):
                w = cst.tile([128, 1024], F32, tag=f"wq_t{p}")
                nc.sync.dma_start(w[:], wq[p][:])
                wq_t.append(w)
                w = cst.tile([128, 1024], F32, tag=f"wo_t{p}")
                nc.sync.dma_start(w[:], wo[p][:])
                wo_t.append(w)
                w = cst.tile([128, 1], F32, tag=f"bq_t{p}")
                nc.sync.dma_start(w[:], bq[p][:])
                bq_t.append(w)
            xt_t = []
            for t in range(KT):
                w = big.tile([128, S], F32, tag=f"xt{t}")
                nc.sync.dma_start(w[:], xt[t * 128:(t + 1) * 128, :])
                xt_t.append(w)

            # ---- phase 1: qT projection (pair-stacked) ----
            qt_t = []
            for p in range(2):
                pq = ps_big.tile([128, S], F32, tag="bigps", name=f"pq{p}")
                for t in range(KT):
                    for c in range(4):
                        nc.tensor.matmul(
                            pq[:, c * 512:(c + 1) * 512],
                            wq_t[p][:, t * 128:(t + 1) * 128],
                            xt_t[t][:, c * 512:(c + 1) * 512],
                            start=(t == 0), stop=(t == KT - 1),
                        )
                q = big.tile([128, S], F32, tag=f"qt{p}")
                nc.scalar.activation(q[:], pq[:], AF.Identity,
                                     bias=bq_t[p][:], scale=1.0)
                qt_t.append(q)

            # ---- phase 2: v tiles (transposed q + ones col) ----
            v_t = []
            for h in range(HPC):
                p, r = h // 2, (h % 2) * 64
                vt = big.tile([128, QT * 65], F32, tag=f"v{h}")
                for half in range(2):
                    pv = ps_small.tile([128, 1024], F32, tag="smallps", name=f"pv{h}_{half}")
                    for tt in range(8):
                        i = half * 8 + tt
                        nc.tensor.transpose(
                            pv[:, tt * 64:(tt + 1) * 64],
                            qt_t[p][r:r + 64, i * 128:(i + 1) * 128],
                            id_t[r:r + 64, :],
                        )
                    dst = (vt[:, half * 520:half * 520 + 520]
                           .rearrange("p (t e) -> p t e", e=65)[:, :, 0:64])
                    src = pv[:, 0:512].rearrange("p (t e) -> p t e", e=64)
                    nc.vector.tensor_copy(dst, src)
                ones = vt[:].rearrange("p (t e) -> p t e", e=65)[:, :, 64:65]
                nc.vector.memset(ones, 1.0)
                v_t.append(vt)

            # ---- phase 3: per-head scores -> floor -> exp -> AV ----
            oTn_t = []
            for p in range(2):
                w = big.tile([128, S], F32, tag=f"oTn{p}")
                oTn_t.append(w)
            for h in range(HPC):
                p, r = h // 2, (h % 2) * 64
                qv = qt_t[p][r:r + 64, :]
                poT = ps_big.tile([65, S], F32, tag="bigps", name=f"poT{h}")
                for i in range(QT):
                    ntile = wrk.tile([128, S], F32, tag="ntile")
                    for j in range(2):
                        pss = ps_small.tile([128, 1024], F32, tag="smallps", name=f"pss{h}_{i}_{j}")
                        for c in range(2):
                            nc.tensor.matmul(
                                pss[:, c * 512:(c + 1) * 512],
                                qv[:, i * 128:(i + 1) * 128],
                                qv[:, j * 1024 + c * 512:j * 1024 + (c + 1) * 512],
                                start=True, stop=True,
                            )
                        nc.vector.tensor_scalar(
                            ntile[:, j * 1024:(j + 1) * 1024], pss[:],
                            63.5, C23, ADD, ADD,
                        )
                    ptile = wrk.tile([128, S], F32, tag="ptile")
                    nc.scalar.activation(ptile[:], ntile[:], AF.Exp,
                                         bias=b_negc[:], scale=1.0)
                    for c in range(4):
                        nc.tensor.matmul(
                            poT[:, c * 512:(c + 1) * 512],
                            v_t[h][:, i * 65:(i + 1) * 65],
                            ptile[:, c * 512:(c + 1) * 512],
                            start=(i == 0), stop=(i == QT - 1),
                        )
                # normalization: rz = exp(-ln Z), broadcast, multiply
                lnz = zs.tile([1, S], F32, tag="lnz")
                nc.scalar.activation(lnz[:], poT[64:65, :], AF.Ln,
                                     bias=0.0, scale=1.0)
                rz = zs.tile([1, S], F32, tag="rz")
                nc.scalar.activation(rz[:], lnz[:], AF.Exp,
                                     bias=0.0, scale=-1.0)
                nc.sync.dma_start(rzscr[h:h + 1, :], rz[:])
                repz = zs.tile([64, S], F32, tag="repz")
                nc.sync.dma_start(repz[:], rzscr[h:h + 1, :].broadcast_to([64, S]))
                nc.vector.tensor_tensor(oTn_t[p][r:r + 64, :], poT[0:64, :],
                                        repz[:], MULT)

            # ---- phase 4: output projection ----
            for m in range(QT):
                po = ps_small.tile([128, 1024], F32, tag="smallps", name=f"po{m}")
                for c in range(2):
                    nc.tensor.matmul(
                        po[:, c * 512:(c + 1) * 512],
                        oTn_t[0][:, m * 128:(m + 1) * 128],
                        wo_t[0][:, c * 512:(c + 1) * 512],
                        start=True, stop=False,
                    )
                    nc.tensor.matmul(
                        po[:, c * 512:(c + 1) * 512],
                        oTn_t[1][:, m * 128:(m + 1) * 128],
                        wo_t[1][:, c * 512:(c + 1) * 512],
                        start=False, stop=True,
                    )
                ot = wrk.tile([128, S], F32, tag="ntile", name=f"ostage{m}")
                if m % 2 == 0:
                    nc.vector.tensor_copy(ot[:, 0:1024], po[:])
                else:
                    nc.scalar.copy(ot[:, 0:1024], po[:])
                nc.sync.dma_start(part[m * 128:(m + 1) * 128, :], ot[:, 0:1024])

    nc.finalize()
    return nc


def _get_nc():
    global _NC_CACHE
    if _NC_CACHE is None:
        _NC_CACHE = _build()
    return _NC_CACHE


def kernel(x, Wq, bq, Wo, bo):
    x = np.asarray(x, np.float32)
    Wq = np.asarray(Wq, np.float32)
    bq = np.asarray(bq, np.float32)
    Wo = np.asarray(Wo, np.float32)
    bo = np.asarray(bo, np.float32)

    eye = np.eye(64, dtype=np.float32)
    ident = np.vstack([eye, eye])
    in_maps = []
    for c in range(NCORES):
        b, hb = c // 4, (c % 4) * HPC
        m = {
            "xt": np.ascontiguousarray(x[b].T) * np.float32(SQ8),
            "ident": ident,
        }
        for p in range(2):
            lo = (hb + 2 * p) * HD          # first col/row of this head pair
            wq_cols = Wq[:, lo:lo + 128]    # [1024, 128]
            # lhsT k-tile layout: [128 part, 8 ktiles x 128]
            m[f"wq{p}"] = np.ascontiguousarray(
                wq_cols.reshape(KT, 128, 128).transpose(1, 0, 2).reshape(128, 1024)
            )
            m[f"wo{p}"] = np.ascontiguousarray(Wo[lo:lo + 128, :]) * np.float32(1.0 / SQ8)
            m[f"bq{p}"] = (bq[lo:lo + 128, None] * np.float32(SQ8)).astype(np.float32)
        in_maps.append(m)

    res = run_bass_kernel_spmd(_get_nc(), in_maps, list(range(NCORES)))
    parts = [r["part"] for r in res.results]
    out = np.empty((B, S, HID), np.float32)
    for b in range(B):
        out[b] = parts[4 * b] + parts[4 * b + 1] + parts[4 * b + 2] + parts[4 * b + 3]
        out[b] += bo[None, :]
    return out


# revision 8
# speedup vs baseline: 2.0962x; 2.0962x over previous
"""Multi-head attention (shared QKV projection, floor-div scores) on 8 NeuronCores.

Problem: B=2, S=2048, HID=1024, NH=16, HD=64
    q = k = v = x @ Wq + bq          (reshaped to heads)
    scores = floor(q k^T / sqrt(64)) ; attn = softmax(scores)
    out = (attn v) @ Wo + bo

Sharding: core c handles batch c//4 and 4 heads ((c%4)*4 ..+4). Each core
computes its heads' contribution to out[b] = attn_out @ Wo; the host sums the
4 partials per batch and adds bo.

Device algorithm per core (all fp32):
  - host pre-scales xT by 8^-0.5 (and bq by 8^-0.5, Wo by 8^0.5) so the PE
    score matmuls directly produce s/8 = scores/sqrt(HD).
  - qT[2 head-pair tiles, 128x2048] = Wq_pair^T @ xT   (PE, PSUM accum)
  - v tiles = PE-transposed qT slices (+ ones column for rowsum Z)
  - per head: scoresT blocks [128k x 2048q] (PE) ->
      floor via RNE trick on DVE: n = (s/8 + 63.5) + 2^23  ->
      P = exp(n - (2^23+64)) on ACT ->
      oT[65, 2048] += v_i^T @ P_i  (PE, ones col gives Z row)
    then rz = exp(-ln(Z)) (ACT), broadcast via DRAM round-trip DMA,
    oTn = oT * rz (DVE)
  - partial = oTn_pair^T @ Wo_pair (PE, K=128 per pair) -> DMA PSUM->DRAM
"""

import math
import sys

sys.path.insert(0, "/opt/trn_rl_repo")

import numpy as np
import concourse.bass as bass
import concourse.bacc as bacc
import concourse.tile as tile
from concourse import mybir
from concourse.bass_utils import run_bass_kernel_spmd

F32 = mybir.dt.float32
F16 = mybir.dt.float16
ADD = mybir.AluOpType.add
MULT = mybir.AluOpType.mult
AF = mybir.ActivationFunctionType

B, S, HID, NH, HD = 2, 2048, 1024, 16, 64
HPC = 4          # heads per core
NCORES = 8
KT = HID // 128  # 8 k-tiles
QT = S // 128    # 16 q/s tiles
C23 = float(2 ** 23)
PSHIFT = 10.0   # P = e^(n-PSHIFT); cancels in softmax; keeps P < fp16 max
SQ8 = 1.0 / math.sqrt(8.0)

_NC_CACHE = None


def _build():
    nc = bacc.Bacc("TRN2", target_bir_lowering=False, debug=False,
                   num_devices=NCORES)

    xt = nc.dram_tensor("xt", [HID, S], F16, kind="ExternalInput")
    wq = [nc.dram_tensor(f"wq{p}", [128, 1024], F16, kind="ExternalInput")
          for p in range(2)]
    wo = [nc.dram_tensor(f"wo{p}", [128, 1024], F16, kind="ExternalInput")
          for p in range(2)]
    bq = [nc.dram_tensor(f"bq{p}", [128, 1], F32, kind="ExternalInput")
          for p in range(2)]
    ident = nc.dram_tensor("ident", [128, 64], F16, kind="ExternalInput")
    part = nc.dram_tensor("part", [S, HID], F16, kind="ExternalOutput")
    rzscr = nc.dram_tensor("rzscr", [HPC, S], F32)

    with tile.TileContext(nc) as tc:
        with (
            tc.tile_pool(name="cst", bufs=1) as cst,
            tc.tile_pool(name="big", bufs=1) as big,
            tc.tile_pool(name="wrk", bufs=3) as wrk,
            tc.tile_pool(name="zs", bufs=1) as zs,
            tc.tile_pool(name="ps_big", bufs=1, space="PSUM") as ps_big,
            tc.tile_pool(name="ps_small", bufs=2, space="PSUM") as ps_small,
        ):
            # ---- constants / inputs ----
            b_negc = cst.tile([128, 1], F32, tag="b_negc")
            nc.vector.memset(b_negc[:], -(C23 + 64.0 + PSHIFT))
            id_t = cst.tile([128, 64], F16, tag="id_t")
            nc.sync.dma_start(id_t[:], ident[:])
            wq_t, wo_t, bq_t = [], [], []
            for p in range(2):
                w = cst.tile([128, 1024], F16, tag=f"wq_t{p}")
                nc.sync.dma_start(w[:], wq[p][:])
                wq_t.append(w)
                w = cst.tile([128, 1024], F16, tag=f"wo_t{p}")
                nc.sync.dma_start(w[:], wo[p][:])
                wo_t.append(w)
                w = cst.tile([128, 1], F32, tag=f"bq_t{p}")
                nc.sync.dma_start(w[:], bq[p][:])
                bq_t.append(w)
            xt_t = []
            for t in range(KT):
                w = big.tile([128, S], F16, tag=f"xt{t}")
                nc.sync.dma_start(w[:], xt[t * 128:(t + 1) * 128, :])
                xt_t.append(w)

            # ---- phase 1: qT projection (pair-stacked) ----
            qt_t = []
            for p in range(2):
                pq = ps_big.tile([128, S], F32, tag="bigps", name=f"pq{p}")
                for t in range(KT):
                    for c in range(4):
                        nc.tensor.matmul(
                            pq[:, c * 512:(c + 1) * 512],
                            wq_t[p][:, t * 128:(t + 1) * 128],
                            xt_t[t][:, c * 512:(c + 1) * 512],
                            start=(t == 0), stop=(t == KT - 1),
                        )
                q = big.tile([128, S], F16, tag=f"qt{p}")
                nc.scalar.activation(q[:], pq[:], AF.Identity,
                                     bias=bq_t[p][:], scale=1.0)
                qt_t.append(q)

            # ---- phase 2: v tiles (transposed q + ones col) ----
            v_t = []
            for h in range(HPC):
                p, r = h // 2, (h % 2) * 64
                vt = big.tile([128, QT * 65], F16, tag=f"v{h}")
                for half in range(2):
                    pv = ps_small.tile([128, 512], F16, tag="smallps", name=f"pv{h}_{half}")
                    for tt in range(8):
                        i = half * 8 + tt
                        nc.tensor.transpose(
                            pv[:, tt * 64:(tt + 1) * 64],
                            qt_t[p][r:r + 64, i * 128:(i + 1) * 128],
                            id_t[r:r + 64, :],
                        )
                    dst = (vt[:, half * 520:half * 520 + 520]
                           .rearrange("p (t e) -> p t e", e=65)[:, :, 0:64])
                    src = pv[:, 0:512].rearrange("p (t e) -> p t e", e=64)
                    nc.vector.tensor_copy(dst, src)
                ones = vt[:].rearrange("p (t e) -> p t e", e=65)[:, :, 64:65]
                nc.vector.memset(ones, 1.0)
                v_t.append(vt)

            # ---- phase 3: per-head scores -> floor -> exp -> AV ----
            oTn_t = []
            for p in range(2):
                w = big.tile([128, S], F16, tag=f"oTn{p}")
                oTn_t.append(w)
            for h in range(HPC):
                p, r = h // 2, (h % 2) * 64
                qv = qt_t[p][r:r + 64, :]
                poT = ps_big.tile([65, S], F32, tag="bigps", name=f"poT{h}")
                for i in range(QT):
                    ntile = wrk.tile([128, S], F32, tag="ntile")
                    for j in range(2):
                        pss = ps_small.tile([128, 1024], F32, tag="smallps", name=f"pss{h}_{i}_{j}")
                        for c in range(2):
                            nc.tensor.matmul(
                                pss[:, c * 512:(c + 1) * 512],
                                qv[:, i * 128:(i + 1) * 128],
                                qv[:, j * 1024 + c * 512:j * 1024 + (c + 1) * 512],
                                start=True, stop=True,
                            )
                        nc.vector.tensor_scalar(
                            ntile[:, j * 1024:(j + 1) * 1024], pss[:],
                            63.5, C23, ADD, ADD,
                        )
                    ptile = wrk.tile([128, S], F16, tag="ptile")
                    nc.scalar.activation(ptile[:], ntile[:], AF.Exp,
                                         bias=b_negc[:], scale=1.0)
                    for c in range(4):
                        nc.tensor.matmul(
                            poT[:, c * 512:(c + 1) * 512],
                            v_t[h][:, i * 65:(i + 1) * 65],
                            ptile[:, c * 512:(c + 1) * 512],
                            start=(i == 0), stop=(i == QT - 1),
                        )
                # normalization: rz = exp(-ln Z), broadcast, multiply
                lnz = zs.tile([1, S], F32, tag="lnz")
                nc.scalar.activation(lnz[:], poT[64:65, :], AF.Ln,
                                     bias=0.0, scale=1.0)
                rz = zs.tile([1, S], F32, tag="rz")
                nc.scalar.activation(rz[:], lnz[:], AF.Exp,
                                     bias=0.0, scale=-1.0)
                nc.sync.dma_start(rzscr[h:h + 1, :], rz[:])
                repz = zs.tile([64, S], F32, tag="repz")
                nc.sync.dma_start(repz[:], rzscr[h:h + 1, :].broadcast_to([64, S]))
                nc.vector.tensor_tensor(oTn_t[p][r:r + 64, :], poT[0:64, :],
                                        repz[:], MULT)

            # ---- phase 4: output projection ----
            for m in range(QT):
                po = ps_small.tile([128, 1024], F32, tag="smallps", name=f"po{m}")
                for c in range(2):
                    nc.tensor.matmul(
                        po[:, c * 512:(c + 1) * 512],
                        oTn_t[0][:, m * 128:(m + 1) * 128],
                        wo_t[0][:, c * 512:(c + 1) * 512],
                        start=True, stop=False,
                    )
                    nc.tensor.matmul(
                        po[:, c * 512:(c + 1) * 512],
                        oTn_t[1][:, m * 128:(m + 1) * 128],
                        wo_t[1][:, c * 512:(c + 1) * 512],
                        start=False, stop=True,
                    )
                ot = wrk.tile([128, 1024], F16, tag="ostage", name=f"ostage{m}")
                if m % 2 == 0:
                    nc.vector.tensor_copy(ot[:], po[:])
                else:
                    nc.scalar.copy(ot[:], po[:])
                nc.sync.dma_start(part[m * 128:(m + 1) * 128, :], ot[:])

    nc.finalize()
    return nc


def _get_nc():
    global _NC_CACHE
    if _NC_CACHE is None:
        _NC_CACHE = _build()
    return _NC_CACHE


def kernel(x, Wq, bq, Wo, bo):
    x = np.asarray(x, np.float32)
    Wq = np.asarray(Wq, np.float32)
    bq = np.asarray(bq, np.float32)
    Wo = np.asarray(Wo, np.float32)
    bo = np.asarray(bo, np.float32)

    eye = np.eye(64, dtype=np.float32)
    ident = np.vstack([eye, eye])
    in_maps = []
    for c in range(NCORES):
        b, hb = c // 4, (c % 4) * HPC
        m = {
            "xt": (np.ascontiguousarray(x[b].T) * np.float32(SQ8)).astype(np.float16),
            "ident": ident.astype(np.float16),
        }
        for p in range(2):
            lo = (hb + 2 * p) * HD          # first col/row of this head pair
            wq_cols = Wq[:, lo:lo + 128]    # [1024, 128]
            # lhsT k-tile layout: [128 part, 8 ktiles x 128]
            m[f"wq{p}"] = np.ascontiguousarray(
                wq_cols.reshape(KT, 128, 128).transpose(1, 0, 2).reshape(128, 1024)
            ).astype(np.float16)
            m[f"wo{p}"] = (np.ascontiguousarray(Wo[lo:lo + 128, :]) * np.float32(1.0 / SQ8)).astype(np.float16)
            m[f"bq{p}"] = (bq[lo:lo + 128, None] * np.float32(SQ8)).astype(np.float32)
        in_maps.append(m)

    res = run_bass_kernel_spmd(_get_nc(), in_maps, list(range(NCORES)))
    parts = [r["part"] for r in res.results]
    out = np.empty((B, S, HID), np.float32)
    for b in range(B):
        out[b] = (parts[4 * b].astype(np.float32) + parts[4 * b + 1].astype(np.float32)
                  + parts[4 * b + 2].astype(np.float32) + parts[4 * b + 3].astype(np.float32))
        out[b] += bo[None, :]
    return out
